# revision 9
# baseline (speedup 1.0000x reference)
"""Trainium2 Bass kernel for nn_BackBone (LSTM backbone + fc + outer-product head).

Data-parallel over batch across 8 NeuronCores. Per core (b_loc rows), v2:
  - history transposed + cast to fp16 on the HOST: xt[D+1, T, B] with a
    constant-1.0 feature row appended so the gate bias rides the projection
    matmul (no PE transposes, no SWDGE cast, half the input HBM traffic)
  - all DMA on HWDGE queues (sync for loads, scalar for stores); GpSimd is
    free and picks up part of the tail einsum
  - two 512-col batch chains; per chain-step the 4 gate preactivations live
    in two [128,1024] PSUM tiles (i|f and g|o) so sigmoid runs merged over
    i,f in one ACTIVATE; all 8 PSUM banks are gate banks (y2 head borrows
    slices in the prologue before the first projection)
  - PE stream per step: recA, recB, projA(t+1), projB(t+1) — recurrent
    matmuls accumulate onto the pre-computed projections (stop=True)
  - output stored fp16 (host casts to fp32): halves output traffic
  - head einsum on DVE during the recurrence (y2 half) and DVE+GpSimd at the
    tail (y1 half); final h transposed b-major via X-bar DMA transpose
"""
import numpy as np

import concourse.bacc as bacc
import concourse.mybir as mybir
import concourse.tile as tile
from concourse import bass_utils

F32 = mybir.dt.float32
F16 = mybir.dt.float16
AF = mybir.ActivationFunctionType

T = 20
D = 340
DP = D + 1               # +1 constant feature row carrying the gate bias
H = 128
E = 32
L = 10
M3 = 3
DCH = [(0, 128), (128, 256), (256, DP)]   # contraction chunks of DP
N_CORES = 8


def build_program(b_loc: int):
    assert b_loc % 256 == 0
    NJ = b_loc // 128
    CW = b_loc // 2               # chain width (<= 512)
    assert CW <= 512
    NCB = 2
    # x DMA t-groups: first small so step 0 starts early
    TGR = ([(0, 2), (2, 8), (8, 14), (14, 20)] if T == 20 else [(0, T)])

    nc = bacc.Bacc("TRN2", target_bir_lowering=False, debug=False)
    xt_d = nc.dram_tensor("xt", (DP, T, b_loc), F16, kind="ExternalInput").ap()
    cnt_d = nc.dram_tensor("cn_t", (E, b_loc), F16, kind="ExternalInput").ap()
    pref_d = nc.dram_tensor("pref_g", (128, NJ, L, M3), F16,
                            kind="ExternalInput").ap()
    wih_d = nc.dram_tensor("w_ih4", (DP, 4 * H), F16, kind="ExternalInput").ap()
    whh_d = nc.dram_tensor("w_hh_t", (H, 4 * H), F16, kind="ExternalInput").ap()
    fcw_d = nc.dram_tensor("fc_w_t", (E, H), F16, kind="ExternalInput").ap()
    fcb_d = nc.dram_tensor("fc_b_row", (1, H), F16, kind="ExternalInput").ap()
    ones_d = nc.dram_tensor("ones_row", (1, 128), F16, kind="ExternalInput").ap()
    out = nc.dram_tensor("out", (b_loc, L * 256 * M3), F16,
                         kind="ExternalOutput").ap()

    with tile.TileContext(nc) as tc:
        with tc.tile_pool(name="wpool", bufs=1) as wpool, \
             tc.tile_pool(name="main", bufs=1) as pool, \
             tc.tile_pool(name="psum", bufs=1, space="PSUM") as pspool:

            # ---- weights / constants ----
            wih_t = []
            for k, (c0, c1) in enumerate(DCH):
                wt_ = wpool.tile([c1 - c0, 4 * H], F16, name=f"wih{k}")
                nc.sync.dma_start(wt_[:], wih_d[c0:c1, :])
                wih_t.append(wt_)
            whh_t = wpool.tile([H, 4 * H], F16, name="whh_t")
            nc.sync.dma_start(whh_t[:], whh_d)
            cnt_t = wpool.tile([E, b_loc], F16, name="cnt_t")
            nc.sync.dma_start(cnt_t[:], cnt_d)
            fcw_t = wpool.tile([E, H], F16, name="fcw_t")
            nc.sync.dma_start(fcw_t[:], fcw_d)
            fcb_t = wpool.tile([1, H], F16, name="fcb_t")
            nc.sync.dma_start(fcb_t[:], fcb_d)
            ones_t = wpool.tile([1, 128], F16, name="ones_t")
            nc.sync.dma_start(ones_t[:], ones_d)
            pf_t = wpool.tile([128, NJ, L, M3], F16, name="pf_t")
            nc.sync.dma_start(pf_t[:], pref_d)

            # ---- persistent fp16 xT tiles, loaded in t-groups ----
            xt_tiles = []
            for k, (c0, c1) in enumerate(DCH):
                xt_tiles.append(
                    pool.tile([c1 - c0, T, b_loc], F16, name=f"xt{k}",
                              tag=f"xt{k}"))
            for (t0, t1) in TGR:
                for k, (c0, c1) in enumerate(DCH):
                    nc.sync.dma_start(xt_tiles[k][:, t0:t1, :],
                                      xt_d[c0:c1, t0:t1, :])

            # ---- PSUM: all 8 banks as gate tiles; one gate per BANK (a
            # start=True matmul invalidates any other in-flight accumulation
            # group sharing its bank) ----
            P1 = [pspool.tile([128, 2, 512], F32, name=f"p1_{cb}",
                              tag=f"p1_{cb}") for cb in range(NCB)]
            P2 = [pspool.tile([128, 2, 512], F32, name=f"p2_{cb}",
                              tag=f"p2_{cb}") for cb in range(NCB)]

            out3 = out.rearrange("p (l nm) -> p l nm", l=L)

            store_jobs = []          # deferred einsum+store emitters

            def emit_einsum(j, y_half, n_off, l0, nl, engine):
                rows = j * 128
                ol = pool.tile([128, nl, 128, M3], F16, name="ol",
                               tag="outl", bufs=6)
                y_b = y_half[:, None, :, None].broadcast_to([128, nl, 128, M3])
                p_b = pf_t[:, j, l0:l0 + nl, None, :].broadcast_to(
                    [128, nl, 128, M3])
                engine.tensor_mul(ol[:], y_b, p_b)
                nc.scalar.dma_start(
                    out3[rows:rows + 128, l0:l0 + nl,
                         n_off * 3:n_off * 3 + 384], ol[:])

            # ---- y2 head (prologue): borrows P1[0] slices before t=0 ----
            y2b = []
            for j in range(NJ):
                tgt = P1[0][:, j % 2, (j // 2) * 128:(j // 2) * 128 + 128]
                nc.tensor.matmul(tgt, cnt_t[:, j * 128:(j + 1) * 128],
                                 fcw_t[:], start=True, stop=False)
                nc.tensor.matmul(tgt, ones_t[:], fcb_t[:],
                                 start=False, stop=True)
                yb = pool.tile([128, 128], F16, name="y2b", tag="y2b",
                               bufs=NJ)
                nc.scalar.activation(yb[:], tgt, AF.Relu)
                y2b.append(yb)
                for l0 in range(0, L, 5):
                    store_jobs.append((j, yb, 128, l0, 5))

            def emit_proj(t, cb, stop):
                cs = slice(cb * CW, (cb + 1) * CW)
                for k in range(3):
                    # gate order i,f (P1) then g,o (P2)
                    for g, ptile in ((0, P1[cb]), (1, P1[cb]),
                                     (2, P2[cb]), (3, P2[cb])):
                        nc.tensor.matmul(
                            ptile[:, g % 2, 0:CW],
                            wih_t[k][:, g * 128:(g + 1) * 128],
                            xt_tiles[k][:, t, cs],
                            start=(k == 0), stop=(stop and k == 2))

            def emit_rec(cb, h_prev):
                for g, ptile in ((0, P1[cb]), (1, P1[cb]),
                                 (2, P2[cb]), (3, P2[cb])):
                    nc.tensor.matmul(ptile[:, g % 2, 0:CW],
                                     whh_t[:, g * 128:(g + 1) * 128],
                                     h_prev[:],
                                     start=False, stop=True)

            # per-chain state tiles
            def new_state(tag):
                return pool.tile([128, CW], F16, name=tag, tag=tag, bufs=2)

            h_prev = [None, None]
            c_prev = [None, None]

            # ---- prologue projections for t=0 ----
            emit_proj(0, 0, stop=True)
            emit_proj(0, 1, stop=True)

            # ---- recurrence ----
            job_i = 0
            for t in range(T):
                # PE: recurrent matmuls accumulate onto projections
                if t > 0:
                    emit_rec(0, h_prev[0])
                    emit_rec(1, h_prev[1])

                gif = [pool.tile([128, 2, CW], F16, name="gif",
                                 tag=f"gif{cb}", bufs=2) for cb in range(NCB)]
                gg = [new_state(f"gg{cb}") for cb in range(NCB)]
                go = [new_state(f"go{cb}") for cb in range(NCB)]
                c_t = ([new_state(f"c{cb}") for cb in range(NCB)]
                       if t > 0 else [None, None])
                tc_t = [new_state(f"tc{cb}") for cb in range(NCB)]
                h_t = [new_state(f"h{cb}") for cb in range(NCB)]
                t1 = ([new_state(f"t1{cb}") for cb in range(NCB)]
                      if t > 0 else [None, None])
                t2 = [new_state(f"t2{cb}") for cb in range(NCB)]

                # chain A activations + cell
                nc.scalar.activation(gif[0][:], P1[0][:, :, 0:CW], AF.Sigmoid)
                nc.scalar.activation(gg[0][:], P2[0][:, 0, 0:CW], AF.Tanh)
                nc.scalar.activation(go[0][:], P2[0][:, 1, 0:CW], AF.Sigmoid)
                if t > 0:
                    nc.vector.tensor_mul(t1[0][:], gif[0][:, 1, :],
                                         c_prev[0][:])
                nc.vector.tensor_mul(t2[0][:], gif[0][:, 0, :], gg[0][:])
                if t > 0:
                    nc.vector.tensor_add(c_t[0][:], t1[0][:], t2[0][:])
                else:
                    c_t[0] = t2[0]
                # chain B activations start while A's cell math runs
                nc.scalar.activation(gif[1][:], P1[1][:, :, 0:CW], AF.Sigmoid)
                nc.scalar.activation(gg[1][:], P2[1][:, 0, 0:CW], AF.Tanh)
                if t > 0:
                    nc.vector.tensor_mul(t1[1][:], gif[1][:, 1, :],
                                         c_prev[1][:])
                nc.scalar.activation(tc_t[0][:], c_t[0][:], AF.Tanh)
                nc.vector.tensor_mul(t2[1][:], gif[1][:, 0, :], gg[1][:])
                if t > 0:
                    nc.vector.tensor_add(c_t[1][:], t1[1][:], t2[1][:])
                else:
                    c_t[1] = t2[1]
                nc.vector.tensor_mul(h_t[0][:], go[0][:], tc_t[0][:])
                nc.scalar.activation(go[1][:], P2[1][:, 1, 0:CW], AF.Sigmoid)
                nc.scalar.activation(tc_t[1][:], c_t[1][:], AF.Tanh)
                nc.vector.tensor_mul(h_t[1][:], go[1][:], tc_t[1][:])

                # PE: projections for t+1
                if t + 1 < T:
                    emit_proj(t + 1, 0, stop=False)
                    emit_proj(t + 1, 1, stop=False)

                h_prev = h_t
                c_prev = c_t

                # sprinkle y2 einsum+stores through the recurrence (DVE)
                if t >= 1:
                    budget = 1 if t < T - 1 else len(store_jobs) - job_i
                    for _ in range(budget):
                        if job_i >= len(store_jobs):
                            break
                        j, yb, n_off, l0, nl = store_jobs[job_i]
                        emit_einsum(j, yb, n_off, l0, nl, nc.vector)
                        job_i += 1

            # ---- tail: y1 half ----
            NJH = NJ // 2
            for j in range(NJ):
                y1 = pool.tile([128, 128], F16, name="y1b", tag="y1b", bufs=4)
                src = h_prev[j // NJH][:, (j % NJH) * 128:(j % NJH) * 128 + 128]
                nc.sync.dma_start_transpose(y1[:], src)
                eng = nc.gpsimd if j in (2, 5, 7) else nc.vector
                for l0 in range(0, L, 5):
                    emit_einsum(j, y1, 0, l0, 5, eng)

    nc.compile()
    return nc


def prep_in_maps(inputs, n_cores: int, b_loc: int):
    history = np.asarray(inputs["history"], np.float32)
    cluster = np.asarray(inputs["cluster_num"], np.float32)
    pref = np.asarray(inputs["pref"], np.float32)
    w_ih = np.asarray(inputs["W_ih"], np.float32)
    w_hh = np.asarray(inputs["W_hh"], np.float32)
    b_ih = np.asarray(inputs["b_ih"], np.float32)
    b_hh = np.asarray(inputs["b_hh"], np.float32)
    fc_w = np.asarray(inputs["fc_w"], np.float32)
    fc_b = np.asarray(inputs["fc_b"], np.float32)

    NJ = b_loc // 128
    w_ih4 = np.concatenate(
        [w_ih.T, (b_ih + b_hh).reshape(1, 4 * H)], axis=0)  # [341, 512]
    shared = {
        "w_ih4": np.ascontiguousarray(w_ih4.astype(np.float16)),
        "w_hh_t": np.ascontiguousarray(w_hh.T.astype(np.float16)),
        "fc_w_t": np.ascontiguousarray(fc_w.T.astype(np.float16)),
        "fc_b_row": np.ascontiguousarray(fc_b.reshape(1, H).astype(np.float16)),
        "ones_row": np.ones((1, 128), np.float16),
    }
    in_maps = []
    for c in range(n_cores):
        r0, r1 = c * b_loc, (c + 1) * b_loc
        hist16 = history[r0:r1].reshape(b_loc, T, D).astype(np.float16)
        xt = np.empty((DP, T, b_loc), np.float16)
        xt[:D] = hist16.transpose(2, 1, 0)
        xt[D] = 1.0
        pref16 = pref[r0:r1].reshape(NJ, 128, L, M3).astype(np.float16)
        in_maps.append({
            "xt": xt,
            "cn_t": np.ascontiguousarray(
                cluster[r0:r1].T.astype(np.float16)),
            "pref_g": np.ascontiguousarray(pref16.transpose(1, 0, 2, 3)),
            **shared,
        })
    return in_maps


def run(inputs, n_cores: int = N_CORES, trace: bool = False):
    B = np.asarray(inputs["history"]).shape[0]
    b_loc = B // n_cores
    nc = build_program(b_loc)
    in_maps = prep_in_maps(inputs, n_cores, b_loc)
    res = bass_utils.run_bass_kernel_spmd(
        nc, in_maps, core_ids=list(range(n_cores)), trace=trace)
    outs = [res.results[c]["out"].astype(np.float32).reshape(
        b_loc, L, 256 * M3) for c in range(n_cores)]
    return np.concatenate(outs, axis=0), res


def kernel(**inputs) -> np.ndarray:
    out, _ = run(inputs, N_CORES)
    return out
